# revision 1
# baseline (speedup 1.0000x reference)
"""Trainium2 Bass kernel for nn_DictlessHeteroLayer (hetero GNN message passing).

  out = sum_r [ x @ W_self[r].T + b_self[r]
                + scatter_add_dst( ew * (x @ W_nei[r].T)[src] ) ]

Strategy (8 NeuronCores, SPMD, no collectives):
  * Host assigns dst nodes to 128-slot tiles (degree balanced, first-fit
    decreasing), deals tiles to cores (edge balanced).  Each core fully owns
    its tiles' output rows -> no cross-core reduction; host re-assembles.
  * Phase 1 (replicated on every core): H[src*4+r, :] = (x @ W_nei[r].T) in
    fp16, written to HBM in relation-interleaved layout (1 KiB contiguous
    runs per partition -> line-rate DMA).  Matmuls in bf16.
  * Phase 2: per PSUM wave (16 dst tiles = 4 banks x 4 quarter-tiles), bulk
    dma_gather of H rows for the wave's edges.  int16 gather indices only
    address 32768 rows, so edges are split into 13 H-row blocks; static
    (tile x block) cells padded to 128-edge chunks (schedule shared by all
    cores = max over cores; pad edges have ew=0).
    Per chunk: DVE builds a one-hot (iota==dstslot)*ew fp16 matrix; TensorE
    accumulates  out[p, d] += sum_e OH[e, p] * Hg[e, d]  into the tile's
    PSUM quarter.  PSUM accumulation groups are per BANK (start=True clears
    has_written for the whole bank).  Self term x @ (sum_r W_self)^T rides
    the same accumulation (float32r); bias is added on the host.
  * Duplicate (rel, src, dst) edges are merged on the host (weights summed,
    exact).  Timing (TimelineSim cost model, per core): ~878 us at ~98%
    DMA-engine utilization (gather + H write are the roofline); rel err vs
    fp32 reference ~1.8e-3 (absmax / output scale), dominated by bf16 x/W
    and fp16 H storage.
"""
import numpy as np

import concourse.bacc as bacc
import concourse.bass as bass
import concourse.mybir as mybir
import concourse.tile as tile
from concourse import bass_utils, library_config

P = 128
D = 128
NREL = 4
NC = 8
BLK = 32768
import os as _os
MAX_CALL_CHUNKS = int(_os.environ.get("KMAXCALL", "32"))
GBUFS = int(_os.environ.get("KGBUFS", "6"))
P1BUFS = int(_os.environ.get("KP1BUFS", "8"))
OHBUFS = int(_os.environ.get("KOHBUFS", "12"))
XHBUFS = int(_os.environ.get("KXHBUFS", "2"))
SPBUFS = int(_os.environ.get("KSPBUFS", "2"))
PH_SLAB = 16              # phase-1 n-tiles per xT slab
WAVE = int(_os.environ.get("KWAVE", "16"))   # dst tiles per PSUM wave


# ----------------------------------------------------------------- scheduling
class Sched:
    pass


def build_schedule(inputs):
    x = np.asarray(inputs["x"], np.float32)
    ei = np.asarray(inputs["edge_index"])
    ew = np.asarray(inputs["edge_weight"], np.float32)
    rel_ptr = np.asarray(inputs["rel_ptr"]).astype(np.int64)
    W_self = np.asarray(inputs["W_self"], np.float32)
    b_self = np.asarray(inputs["b_self"], np.float32)
    W_nei = np.asarray(inputs["W_nei"], np.float32)

    N = x.shape[0]
    E = ei.shape[1]
    NT0 = -(-N // P)
    T_CORE = -(-NT0 // NC)
    NT = T_CORE * NC
    NPAD = NT * P
    HROWS = NREL * NPAD
    NB = -(-HROWS // BLK)
    wave_sizes = []
    j = 0
    while j < T_CORE:
        wave_sizes.append(min(WAVE, T_CORE - j))
        j += WAVE
    NW = len(wave_sizes)

    src = ei[0].astype(np.int64)
    dst = ei[1].astype(np.int64)
    rel = (np.searchsorted(rel_ptr, np.arange(E), side="right") - 1).astype(np.int64)

    # merge duplicate (rel, src, dst) edges (sum their weights) — exact
    ukey = (rel * N + src) * N + dst
    uorder = np.argsort(ukey, kind="stable")
    uk = ukey[uorder]
    first = np.ones(E, bool)
    first[1:] = uk[1:] != uk[:-1]
    gids = np.cumsum(first) - 1
    ew_sum = np.zeros(int(gids[-1]) + 1, np.float64)
    np.add.at(ew_sum, gids, ew[uorder].astype(np.float64))
    keep = uorder[first]
    src, dst, rel = src[keep], dst[keep], rel[keep]
    ew = ew_sum.astype(np.float32)
    E = len(src)

    deg = np.bincount(dst, minlength=N)

    # ---- node -> (tile, slot): first-fit decreasing over NT tiles
    import heapq
    order = np.argsort(-deg, kind="stable")
    tile_of = np.empty(N, np.int64)
    slot_of = np.empty(N, np.int64)
    heap = [(0, t, 0) for t in range(NT)]
    heapq.heapify(heap)
    for n in order:
        load, t, used = heapq.heappop(heap)
        tile_of[n] = t
        slot_of[n] = used
        used += 1
        if used < P:
            heapq.heappush(heap, (load + int(deg[n]), t, used))

    tile_load = np.bincount(tile_of[dst], minlength=NT)

    # ---- tiles -> cores (greedy balance), local index within core
    t_order = np.argsort(-tile_load, kind="stable")
    core_of_tile = np.empty(NT, np.int64)
    local_of_tile = np.empty(NT, np.int64)
    heap = [(0, c, 0) for c in range(NC)]
    heapq.heapify(heap)
    core_fill = [0] * NC
    for t in t_order:
        load, c, cnt = heapq.heappop(heap)
        core_of_tile[t] = c
        local_of_tile[t] = core_fill[c]
        core_fill[c] += 1
        if core_fill[c] < T_CORE:
            heapq.heappush(heap, (load + int(tile_load[t]), c, core_fill[c]))

    # ---- per-edge attributes
    e_tile = tile_of[dst]
    e_core = core_of_tile[e_tile]
    e_j = local_of_tile[e_tile]              # local tile 0..T_CORE-1
    e_w = np.minimum(e_j // WAVE, NW - 1)
    # relation-interleaved H layout: row = src*NREL + r  (write-contiguous)
    gidx = src * NREL + rel
    e_b = gidx // BLK

    # ---- static chunk table C[j, b] = max over cores of ceil(count/128)
    cnt = np.zeros((NC, T_CORE, NB), np.int64)
    np.add.at(cnt, (e_core, e_j, e_b), 1)
    C = -(-cnt.max(axis=0) // P)             # [T_CORE, NB]

    # ---- schedule order: (w, b, slot-ranges); shared by all cores.
    # Within (wave, block): each tile j gets a STATIC slot range of length
    # seg_len[j,b] = max over cores of its edge count (no 128 rounding).
    # Chunks are 128-slot windows of the (w,b) segment (segment end padded to
    # x128); a chunk emits one masked matmul per tile range it intersects.
    seg_len = cnt.max(axis=0)                      # [T_CORE, NB]
    if _os.environ.get("KPAD", "1") == "1":
        # pad each tile range to x128 (fewer matmuls, more gather rows)
        seg_len = -(-seg_len // P) * P
    cell_off = np.zeros((T_CORE, NB), np.int64)    # static slot offset
    off = 0
    npair = 0
    wave_call_plans = []   # per wave: [(b, [(colbase, [chunk -> [[pair,j,stop],...]]), ...])]
    wave_info = []
    w0 = 0
    pair_meta = []         # (pair, chunk_slot0, j, range_lo, range_hi)
    bank_stop_self = []
    for w, wsz in enumerate(wave_sizes):
        jlo, jhi = w0, w0 + wsz
        blocks = []
        wave_p0 = npair
        wave_off0 = off
        wave_mms = []
        for b in range(NB):
            seg0 = off
            ranges = []
            for j in range(jlo, jhi):
                if seg_len[j, b] == 0:
                    continue
                cell_off[j, b] = off
                ranges.append((j, off, off + seg_len[j, b]))
                off += seg_len[j, b]
            seg_edges = off - seg0
            nch = -(-seg_edges // P) if seg_edges else 0
            off = seg0 + nch * P                  # pad segment to x128
            chunk_list = []
            ri = 0
            for k in range(nch):
                c0, c1 = seg0 + k * P, seg0 + (k + 1) * P
                mms = []
                for (j, lo, hi) in ranges:
                    if hi <= c0 or lo >= c1:
                        continue
                    mms.append([npair, j, False])
                    pair_meta.append((npair, c0, j, max(lo, c0), min(hi, c1)))
                    npair += 1
                wave_mms.extend(mms)
                chunk_list.append(mms)
            calls = []
            pos = 0
            while pos < len(chunk_list):
                n = min(MAX_CALL_CHUNKS, len(chunk_list) - pos)
                calls.append(chunk_list[pos : pos + n])
                pos += n
            blocks.append((b, calls))
        # stop flag: last mm per bank
        nbanks = -(-wsz // 4)
        no_chunk_banks = set(range(nbanks))
        seen = {}
        for ent in wave_mms:
            seen[(ent[1] - jlo) // 4] = ent
        for k, ent in seen.items():
            ent[2] = True
            no_chunk_banks.discard(k)
        bank_stop_self.append(no_chunk_banks)
        wave_call_plans.append(blocks)
        wave_info.append(
            dict(w=w, wsz=wsz, jlo=jlo, p0=wave_p0, np=npair - wave_p0,
                 off0=wave_off0)
        )
        w0 += wsz
    NPAIR = npair
    total_slots = off
    CH_TOTAL = total_slots // P

    # ---- per-core flat edge arrays in schedule order
    # rank edges inside each (core, j, b) cell
    key = (e_core * T_CORE + e_j) * NB + e_b
    sort_idx = np.lexsort((gidx, key))
    skey = key[sort_idx]
    newg = np.ones(E, bool)
    newg[1:] = skey[1:] != skey[:-1]
    group_first = np.nonzero(newg)[0]
    group_id = np.cumsum(newg) - 1
    rank = np.arange(E) - group_first[group_id]

    se = sort_idx
    pos_in_core = cell_off[e_j[se], e_b[se]] + rank
    core_se = e_core[se]

    idx_flat = np.zeros((NC, total_slots), np.int16)
    dst_flat = np.zeros((NC, total_slots), np.float32)
    ew_flat = np.zeros((NC, total_slots), np.float32)
    idx_flat[core_se, pos_in_core] = (gidx[se] - e_b[se] * BLK).astype(np.int16)
    dst_flat[core_se, pos_in_core] = slot_of[dst[se]].astype(np.float32)
    ew_flat[core_se, pos_in_core] = ew[se]

    # masked per-(chunk, tile) pair columns [NC, 128, NPAIR]
    dst_dev = np.zeros((NC, P, NPAIR), np.float32)
    ew_dev = np.zeros((NC, P, NPAIR), np.float32)
    for (pr, c0, j, lo, hi) in pair_meta:
        a, bnd = lo - c0, hi - c0
        dst_dev[:, a:bnd, pr] = dst_flat[:, lo:hi]
        ew_dev[:, a:bnd, pr] = ew_flat[:, lo:hi]

    # idx16 wrapped per call: [NC, 128, IDXCOLS]; also rewrite plans to
    # (b, [(colbase, chunklist), ...]) and record per-wave col spans
    call_cols = []
    colbase = 0
    new_plans = []
    wave_colspan = []
    chunk_ctr = 0
    for blocks in wave_call_plans:
        wcb0 = colbase
        nb_list = []
        for b, calls in blocks:
            ncalls = []
            for cl in calls:
                n_idx = len(cl) * P
                slot0 = chunk_ctr * P
                call_cols.append((colbase, slot0, n_idx))
                ncalls.append((colbase, cl))
                chunk_ctr += len(cl)
                colbase += n_idx // 16
            nb_list.append((b, ncalls))
        new_plans.append(nb_list)
        wave_colspan.append((wcb0, colbase))
    wave_call_plans = new_plans
    IDXCOLS = colbase
    idx_dev = np.zeros((NC, P, IDXCOLS), np.int16)
    for cb, slot0, n_idx in call_cols:
        seg = idx_flat[:, slot0 : slot0 + n_idx]            # [NC, n]
        wrap = seg.reshape(NC, n_idx // 16, 16).transpose(0, 2, 1)
        idx_dev[:, :, cb : cb + n_idx // 16] = np.tile(wrap, (1, 8, 1))

    # ---- dense inputs
    import ml_dtypes
    xT = np.zeros((D, NPAD), ml_dtypes.bfloat16)
    xT[:, :N] = x.T.astype(ml_dtypes.bfloat16)
    WT4 = np.empty((D, NREL * D), ml_dtypes.bfloat16)
    for r in range(NREL):
        WT4[:, r * D : (r + 1) * D] = W_nei[r].T.astype(ml_dtypes.bfloat16)
    WselfT = W_self.sum(axis=0).T.copy()               # [k, d]
    bsum = b_self.sum(axis=0).astype(np.float32).reshape(D, 1)
    iotaf = np.tile(np.arange(P, dtype=np.float16), (P, 1))

    # xT_perm per core: [NC, 128, T_CORE*128] column (j*128+p) = x[node(j,p)]
    node_at = np.full((NC, T_CORE, P), -1, np.int64)
    node_at[core_of_tile[tile_of], local_of_tile[tile_of], slot_of] = np.arange(N)
    xtp = np.zeros((NC, D, T_CORE * P), np.float32)
    for c in range(NC):
        nn = node_at[c].reshape(-1)
        valid = nn >= 0
        xtp[c][:, valid] = x[nn[valid]].T

    s = Sched()
    s.N, s.E, s.NPAD, s.NT, s.T_CORE, s.NB, s.NW = N, E, NPAD, NT, T_CORE, NB, NW
    s.HROWS = HROWS
    s.wave_sizes = wave_sizes
    s.wave_call_plans = wave_call_plans
    s.wave_info = wave_info
    s.call_cols = call_cols
    s.wave_colspan = wave_colspan
    s.CH_TOTAL = CH_TOTAL
    s.NPAIR = NPAIR
    s.IDXCOLS = IDXCOLS
    s.seg_len = seg_len
    s.bank_stop_self = bank_stop_self
    s.node_at = node_at
    s.core_of_tile, s.local_of_tile = core_of_tile, local_of_tile
    s.tile_of, s.slot_of = tile_of, slot_of
    s.in_shared = dict(xtr=xT, wt4=WT4, wselft=WselfT, iotaf=iotaf)
    s.bsum = bsum.reshape(-1)
    s.in_percore = [
        dict(idx16=idx_dev[c], dstc=dst_dev[c], ewc=ew_dev[c], xtp=xtp[c])
        for c in range(NC)
    ]
    return s


# ----------------------------------------------------------------- bass build
def build_bass(s, num_devices=NC, repeat=1, phases=(1, 2)):
    f16 = mybir.dt.float16
    f32 = mybir.dt.float32
    f32r = mybir.dt.float32r
    i16 = mybir.dt.int16

    nc = bacc.Bacc("TRN2", num_devices=num_devices)
    xtr = nc.dram_tensor("xtr", [P, s.NPAD], mybir.dt.bfloat16, kind="ExternalInput")
    wt4 = nc.dram_tensor("wt4", [P, NREL * D], mybir.dt.bfloat16, kind="ExternalInput")
    wselft = nc.dram_tensor("wselft", [P, D], f32r, kind="ExternalInput")
    iotaf = nc.dram_tensor("iotaf", [P, P], f16, kind="ExternalInput")
    xtp = nc.dram_tensor("xtp", [P, s.T_CORE * P], f32r, kind="ExternalInput")
    idx16 = nc.dram_tensor("idx16", [P, s.IDXCOLS], i16, kind="ExternalInput")
    dstc = nc.dram_tensor("dstc", [P, s.NPAIR], f32, kind="ExternalInput")
    ewc = nc.dram_tensor("ewc", [P, s.NPAIR], f32, kind="ExternalInput")
    outT = nc.dram_tensor("outT", [s.T_CORE, P, D], f32, kind="ExternalOutput")

    NSLAB = s.NPAD // (PH_SLAB * P)
    assert NSLAB * PH_SLAB * P == s.NPAD

    nc.gpsimd.load_library(library_config.mlp)
    with tile.TileContext(nc) as tc:
        with (
            tc.tile_pool(name="dram", bufs=1, space="DRAM") as dpool,
            tc.tile_pool(name="const", bufs=1) as cpool,
            tc.tile_pool(name="x1", bufs=XHBUFS) as xpool,
            tc.tile_pool(name="hst", bufs=XHBUFS) as hpool,
            tc.tile_pool(name="meta", bufs=2) as mpool,
            tc.tile_pool(name="g", bufs=GBUFS) as gpool,
            tc.tile_pool(name="oh", bufs=OHBUFS) as ohpool,
            tc.tile_pool(name="st", bufs=SPBUFS) as spool,
        ):
            H = dpool.tile([s.HROWS, D], f16)

            wt4_t = cpool.tile([P, NREL * D], mybir.dt.bfloat16)
            nc.sync.dma_start(out=wt4_t[:], in_=wt4[:, :])
            wselft_t = cpool.tile([P, D], f32r)
            nc.sync.dma_start(out=wselft_t[:], in_=wselft[:, :])
            iota_t = cpool.tile([P, P], f16)
            nc.sync.dma_start(out=iota_t[:], in_=iotaf[:, :])

            # ---------------- phase 1: H = x @ W_nei^T (all relations)
            for _rep in range(repeat if 1 in phases else 0):
             with tc.tile_pool(name="p1", bufs=P1BUFS, space="PSUM") as p1pool:
              for sl in range(NSLAB):
                xs = xpool.tile([P, PH_SLAB * P], mybir.dt.bfloat16, tag="xs")
                nc.sync.dma_start(
                    out=xs[:], in_=xtr[:, sl * PH_SLAB * P : (sl + 1) * PH_SLAB * P]
                )
                hs = hpool.tile([P, PH_SLAB, NREL * D], f16, tag="hs")
                for t in range(PH_SLAB):
                    ph = p1pool.tile([P, NREL * D], f32, space="PSUM", tag="ph")
                    nc.tensor.matmul(
                        out=ph[:],
                        lhsT=xs[:, t * P : (t + 1) * P],
                        rhs=wt4_t[:],
                        start=True,
                        stop=True,
                    )
                    if t % 3 == 0:
                        nc.vector.tensor_copy(out=hs[:, t, :], in_=ph[:])
                    else:
                        nc.scalar.copy(out=hs[:, t, :], in_=ph[:])
                # interleaved H: row = src*NREL + r; per-partition runs are
                # (r, d) = 1 KiB contiguous; slab region is one big DMA
                base_row = sl * PH_SLAB * NREL * P
                dram_view = H[base_row : base_row + PH_SLAB * NREL * P, :]
                dram_view = dram_view.rearrange("(t n r) d -> n t r d", r=NREL, n=P)
                nc.sync.dma_start(
                    out=dram_view,
                    in_=hs[:].rearrange("n t (r d) -> n t r d", r=NREL),
                )

            # ---------------- phase 2: waves
            for _rep in range(repeat if 2 in phases else 0):
             with tc.tile_pool(name="p2", bufs=1, space="PSUM") as p2pool:
              for wi, blocks, (wcb0, wcb1) in zip(
                  s.wave_info, s.wave_call_plans, s.wave_colspan
            ):
                w, wsz, jlo, p0 = wi["w"], wi["wsz"], wi["jlo"], wi["p0"]
                npr = wi["np"]
                # wave metadata loads
                idx_w = mpool.tile([P, max(wcb1 - wcb0, 1)], i16, tag="idxw")
                nc.sync.dma_start(out=idx_w[:], in_=idx16[:, wcb0:wcb1])
                dst_w = mpool.tile([P, max(npr, 1)], f32, tag="dstw")
                nc.sync.dma_start(out=dst_w[:], in_=dstc[:, p0 : p0 + npr])
                ew_w = mpool.tile([P, max(npr, 1)], f32, tag="eww")
                nc.sync.dma_start(out=ew_w[:], in_=ewc[:, p0 : p0 + npr])
                xp_w = mpool.tile([P, wsz * P], f32r, tag="xpw")
                nc.sync.dma_start(
                    out=xp_w[:], in_=xtp[:, jlo * P : (jlo + wsz) * P]
                )
                nbanks = -(-wsz // 4)
                banks = []
                for k in range(nbanks):
                    bank_t = p2pool.tile([P, 4 * P], f32, space="PSUM",
                                         tag=f"bank{k}", name=f"bank{k}_w{w}")
                    banks.append(bank_t)

                def quarter(j):
                    jj = j - jlo
                    return banks[jj // 4][:, (jj % 4) * P : (jj % 4 + 1) * P]

                # self matmuls; accumulation group = whole bank: start only on
                # the bank's first matmul, stop on its last (here iff bank
                # has no edge chunks)
                for j in range(jlo, jlo + wsz):
                    jj = j - jlo
                    k = jj // 4
                    last_self_of_bank = (jj % 4 == 3) or (jj == wsz - 1)
                    nc.tensor.matmul(
                        out=quarter(j),
                        lhsT=xp_w[:, jj * P : (jj + 1) * P],
                        rhs=wselft_t[:],
                        start=(jj % 4 == 0),
                        stop=bool(
                            k in s.bank_stop_self[w] and last_self_of_bank
                        ),
                        skip_group_check=True,
                    )
                # gather + one-hot + accumulate
                for b, calls in blocks:
                    lo = b * BLK
                    hi = min(lo + BLK, s.HROWS)
                    for cb, cl in calls:
                        nch = len(cl)
                        g_t = gpool.tile([P, nch, D], f16, tag="g")
                        nc.gpsimd.dma_gather(
                            out_ap=g_t[:],
                            in_ap=H[lo:hi, :],
                            idxs_ap=idx_w[:, cb - wcb0 : cb - wcb0 + nch * 8],
                            num_idxs=nch * P,
                            num_idxs_reg=nch * P,
                            elem_size=D,
                            single_packet=False,
                        )
                        for pos, mms in enumerate(cl):
                            for (pr, j, stop) in mms:
                                oh = ohpool.tile([P, P], f16, tag="oh")
                                nc.vector.tensor_scalar(
                                    out=oh[:],
                                    in0=iota_t[:],
                                    scalar1=dst_w[:, pr - p0 : pr - p0 + 1],
                                    scalar2=ew_w[:, pr - p0 : pr - p0 + 1],
                                    op0=mybir.AluOpType.is_equal,
                                    op1=mybir.AluOpType.mult,
                                )
                                nc.tensor.matmul(
                                    out=quarter(j),
                                    lhsT=oh[:],
                                    rhs=g_t[:, pos, :],
                                    start=False,
                                    stop=stop,
                                    skip_group_check=True,
                                )
                # drain (bias added on host); psum is [p, d] per tile
                stage = spool.tile([P, wsz, P], f32, tag="stage")
                for j in range(jlo, jlo + wsz):
                    nc.vector.tensor_copy(
                        out=stage[:, j - jlo, :], in_=quarter(j)
                    )
                dview = outT[jlo : jlo + wsz, :, :].rearrange("t p d -> p t d")
                nc.sync.dma_start(out=dview, in_=stage[:])
    nc.compile()
    return nc


def kernel(**inputs):
    s = build_schedule(inputs)
    nc = build_bass(s)
    in_maps = []
    for c in range(NC):
        m = dict(s.in_shared)
        m.update(s.in_percore[c])
        in_maps.append(m)
    res = bass_utils.run_bass_kernel_spmd(nc, in_maps, core_ids=list(range(NC)))
    outT = np.stack([res.results[c]["outT"] for c in range(NC)])  # [NC,T,D,P]
    return assemble(s, outT)


def assemble(s, outT):
    N = s.N
    nodes = np.arange(N)
    c = s.core_of_tile[s.tile_of[nodes]]
    t = s.local_of_tile[s.tile_of[nodes]]
    p = s.slot_of[nodes]
    return (outT[c, t, p, :] + s.bsum[None, :]).astype(np.float32)



# revision 5
# speedup vs baseline: 1.9638x; 1.9638x over previous
"""Trainium2 Bass kernel for nn_DictlessHeteroLayer (hetero GNN message passing).

  out = sum_r [ x @ W_self[r].T + b_self[r]
                + scatter_add_dst( ew * (x @ W_nei[r].T)[src] ) ]

Strategy (8 NeuronCores, SPMD, no collectives) — aggregate-first:
  By linearity, scatter_add_dst(ew * (x W_r^T)[src]) = (scatter_add_dst(ew *
  x[src])) W_r^T, so we aggregate RAW x rows per (dst tile, relation) and
  apply W_nei once per (tile, rel) at the end.  This removes the baseline's
  phase-1 H = x@W^T materialization (~128 MB of HBM traffic per core).

  * Host assigns dst nodes to 128-slot tiles (degree-balanced FFD), deals
    tiles to cores (edge balanced).  Each core fully owns its tiles' output
    rows -> no cross-core reduction; host re-assembles.
  * Edges sharded by dst-tile owner.  Per wave of 8 tiles (= 8 PSUM banks,
    bank b_t holds tile t's AGGT: [k=128, 4 rel * 128 dst slots] f32):
    for each of 4 equal 25088-row gather windows, a static (tile, rel)
    sub-cell layout (sizes = max over cores) is packed into 128-edge chunks.
    Per chunk: gpsimd dma_gather of 128 f16 x rows (256 B/row descriptors);
    DVE builds ONE shared one-hot OH[e, (q - qbase)*128 + slot] * ew (f16,
    4x DVE perf mode) covering all quarters the chunk spans; TensorE does one
    quarter-pure matmul per touched (tile, rel):
        AGGT[k, r*128 + p] += sum_e Xg[e, k] * OH[e, col]
    with exact first/last-touch start/stop per quarter.
  * Per tile: Act copies the bank to SBUF f16; the bank's first quarter is
    then reused as out accumulator: out[p, d] = xtp_tile @ WselfSum^T
    + sum_r AGGT_r^T ... via 5 matmuls (lhsT = xtp slice / aggsb quarter,
    rhs = WselfT / W_nei[r]^T, all f16); drained f16 to DRAM (bias added on
    host, output cast to f32 on host).
  * Duplicate (rel, src, dst) edges merged on host (weights summed, exact).
"""
import numpy as np

import concourse.bacc as bacc
import concourse.bass as bass
import concourse.mybir as mybir
import concourse.tile as tile
from concourse import bass_utils, library_config

P = 128
D = 128
NREL = 4
NC = 8
import os as _os
BLKW = 25088              # gather window rows = NPAD / 4
NBLK = 4
WAVE_T = int(_os.environ.get("KWAVET", "8"))    # tiles per wave = psum banks
MAX_CALL = int(_os.environ.get("KMAXCALL", "32"))  # chunks per dma_gather
GBUFS = int(_os.environ.get("KGBUFS", "6"))
OHBUFS = int(_os.environ.get("KOHBUFS", "12"))
ABUFS = int(_os.environ.get("KABUFS", "4"))
SBUFS = int(_os.environ.get("KSBUFS", "2"))
OHW_MAX = 1024            # max one-hot width (8 quarters)


class Sched:
    pass


def build_schedule(inputs):
    import ml_dtypes
    x = np.asarray(inputs["x"], np.float32)
    ei = np.asarray(inputs["edge_index"])
    ew = np.asarray(inputs["edge_weight"], np.float32)
    rel_ptr = np.asarray(inputs["rel_ptr"]).astype(np.int64)
    W_self = np.asarray(inputs["W_self"], np.float32)
    b_self = np.asarray(inputs["b_self"], np.float32)
    W_nei = np.asarray(inputs["W_nei"], np.float32)

    N = x.shape[0]
    E = ei.shape[1]
    NT0 = -(-N // P)
    T_CORE = -(-NT0 // NC)
    NT = T_CORE * NC
    NPAD = NT * P
    assert NPAD == NBLK * BLKW, (NPAD, NBLK * BLKW)

    src = ei[0].astype(np.int64)
    dst = ei[1].astype(np.int64)
    rel = (np.searchsorted(rel_ptr, np.arange(E), side="right") - 1).astype(np.int64)

    # merge duplicate (rel, src, dst) edges (sum their weights) — exact
    ukey = (rel * N + src) * N + dst
    uorder = np.argsort(ukey, kind="stable")
    uk = ukey[uorder]
    first = np.ones(E, bool)
    first[1:] = uk[1:] != uk[:-1]
    gids = np.cumsum(first) - 1
    ew_sum = np.zeros(int(gids[-1]) + 1, np.float64)
    np.add.at(ew_sum, gids, ew[uorder].astype(np.float64))
    keep = uorder[first]
    src, dst, rel = src[keep], dst[keep], rel[keep]
    ew = ew_sum.astype(np.float32)
    E = len(src)

    deg = np.bincount(dst, minlength=N)

    # ---- node -> (tile, slot): first-fit decreasing over NT tiles
    import heapq
    order = np.argsort(-deg, kind="stable")
    tile_of = np.empty(N, np.int64)
    slot_of = np.empty(N, np.int64)
    heap = [(0, t, 0) for t in range(NT)]
    heapq.heapify(heap)
    for n in order:
        load, t, used = heapq.heappop(heap)
        tile_of[n] = t
        slot_of[n] = used
        used += 1
        if used < P:
            heapq.heappush(heap, (load + int(deg[n]), t, used))

    tile_load = np.bincount(tile_of[dst], minlength=NT)

    # ---- tiles -> cores (greedy balance), local index within core
    t_order = np.argsort(-tile_load, kind="stable")
    core_of_tile = np.empty(NT, np.int64)
    local_of_tile = np.empty(NT, np.int64)
    heap = [(0, c, 0) for c in range(NC)]
    heapq.heapify(heap)
    core_fill = [0] * NC
    for t in t_order:
        load, c, cnt_ = heapq.heappop(heap)
        core_of_tile[t] = c
        local_of_tile[t] = core_fill[c]
        core_fill[c] += 1
        if core_fill[c] < T_CORE:
            heapq.heappush(heap, (load + int(tile_load[t]), c, core_fill[c]))

    # ---- per-edge attributes
    e_tile = tile_of[dst]
    e_core = core_of_tile[e_tile]
    e_j = local_of_tile[e_tile]              # local tile 0..T_CORE-1
    e_b = src // BLKW                        # gather window
    e_slot = slot_of[dst]

    # ---- static sub-cell sizes: s[j, r, b] = max over cores
    cnt = np.zeros((NC, T_CORE, NREL, NBLK), np.int64)
    np.add.at(cnt, (e_core, e_j, rel, e_b), 1)
    scell = cnt.max(axis=0)                  # [T_CORE, NREL, NBLK]
    assert scell.reshape(T_CORE, -1).sum(axis=1).min() > 0

    # ---- waves of WAVE_T tiles; per (wave, b) segment: pack sub-cells
    # (j-major, then rel), pad segment to x128.  Chunk = 128 slots.
    waves = []
    j0 = 0
    while j0 < T_CORE:
        waves.append((j0, min(WAVE_T, T_CORE - j0)))
        j0 += WAVE_T
    NW = len(waves)

    cell_off = np.zeros((T_CORE, NREL, NBLK), np.int64)
    off = 0
    # per chunk: (b, qbase, width_q, mms); mms = [(j, r, ohq, start, stop)]
    chunk_meta = []
    wave_plans = []      # per wave: dict(b -> list of calls; call = [chunk ids])
    wave_info = []
    first_touch = {}
    last_touch = {}
    for w, (jlo, wsz) in enumerate(waves):
        wave_ch0 = len(chunk_meta)
        woff0 = off
        blocks = []
        for b in range(NBLK):
            seg0 = off
            ranges = []                      # (q_local, j, r, lo, hi)
            for j in range(jlo, jlo + wsz):
                for r in range(NREL):
                    sz = int(scell[j, r, b])
                    if sz == 0:
                        continue
                    cell_off[j, r, b] = off
                    q = (j - jlo) * NREL + r
                    ranges.append((q, j, r, off, off + sz))
                    off += sz
            seg_edges = off - seg0
            nch = -(-seg_edges // P) if seg_edges else 0
            off = seg0 + nch * P
            # chunks of this segment
            ch_ids = []
            for k in range(nch):
                c0, c1 = seg0 + k * P, seg0 + (k + 1) * P
                touch = [rg for rg in ranges if rg[4] > c0 and rg[3] < c1]
                qbase = touch[0][0] if touch else 0
                qmax = touch[-1][0] if touch else 0
                width_q = qmax - qbase + 1
                assert width_q * P <= OHW_MAX, width_q
                mms = []
                for (q, j, r, lo, hi) in touch:
                    # PSUM semantics: start=True clears has_written for the
                    # WHOLE bank; later matmuls (start=False) init-or-accum
                    # per element.  So start only on the first matmul into
                    # tile j's bank this wave, stop on the last.
                    st = j not in first_touch
                    first_touch.setdefault(j, len(chunk_meta))
                    last_touch[j] = (len(chunk_meta), len(mms))
                    mms.append([j, r, q - qbase, st, False])
                ch_ids.append(len(chunk_meta))
                chunk_meta.append(dict(b=b, qbase=qbase, wq=width_q, mms=mms,
                                       w=w, c0=c0))
            calls = []
            pos = 0
            while pos < len(ch_ids):
                n = min(MAX_CALL, len(ch_ids) - pos)
                calls.append(ch_ids[pos:pos + n])
                pos += n
            blocks.append((b, calls))
        wave_plans.append(blocks)
        wave_info.append(dict(w=w, jlo=jlo, wsz=wsz, ch0=wave_ch0,
                              nch=len(chunk_meta) - wave_ch0, off0=woff0,
                              off1=off))
    # stop flags
    for j, (ci, mi) in last_touch.items():
        chunk_meta[ci]["mms"][mi][4] = True
    # every tile must be touched (else final matmuls read garbage)
    for j in range(T_CORE):
        assert j in first_touch, j
    # every (j, r) quarter must be written at least once (has_written init);
    # quarters with no edges at all would leave stale psum.
    qtouch = set()
    for cm in chunk_meta:
        for (j, r, _, _, _) in cm["mms"]:
            qtouch.add((j, r))
    for j in range(T_CORE):
        for r in range(NREL):
            assert (j, r) in qtouch, (j, r)

    CH = len(chunk_meta)
    total_slots = off
    assert total_slots == CH * P

    # ---- per-core flat edge arrays in schedule order
    key = ((e_core * T_CORE + e_j) * NREL + rel) * NBLK + e_b
    sort_idx = np.lexsort((src, key))
    skey = key[sort_idx]
    newg = np.ones(E, bool)
    newg[1:] = skey[1:] != skey[:-1]
    group_first = np.nonzero(newg)[0]
    group_id = np.cumsum(newg) - 1
    rank = np.arange(E) - group_first[group_id]

    se = sort_idx
    pos_in_core = cell_off[e_j[se], rel[se], e_b[se]] + rank
    core_se = e_core[se]

    idx_flat = np.zeros((NC, total_slots), np.int16)
    dk_flat = np.zeros((NC, total_slots), np.float32)
    ew_flat = np.zeros((NC, total_slots), np.float32)
    idx_flat[core_se, pos_in_core] = (src[se] - e_b[se] * BLKW).astype(np.int16)
    # dstkey = (q_local - qbase_of_chunk)*128 + slot
    q_of_edge = ((e_j[se] - (e_j[se] // WAVE_T) * WAVE_T) * NREL + rel[se])
    ch_of_pos = pos_in_core // P
    qbase_arr = np.asarray([cm["qbase"] for cm in chunk_meta], np.int64)
    dk_flat[core_se, pos_in_core] = (
        (q_of_edge - qbase_arr[ch_of_pos]) * P + e_slot[se]
    ).astype(np.float32)
    ew_flat[core_se, pos_in_core] = ew[se]

    # ---- device metadata
    # dkew: [NC, 128, 2*CH]  (col 2c = dstkey, col 2c+1 = ew)
    dkew = np.zeros((NC, P, 2 * CH), np.float32)
    dk3 = dk_flat.reshape(NC, CH, P).transpose(0, 2, 1)
    ew3 = ew_flat.reshape(NC, CH, P).transpose(0, 2, 1)
    dkew[:, :, 0::2] = dk3
    dkew[:, :, 1::2] = ew3

    # idx16 wrapped per call: [NC, 128, IDXCOLS]
    call_list = []           # (colbase, slot0, n_idx) per call
    colbase = 0
    new_plans = []
    wave_colspan = []
    for w, blocks in enumerate(wave_plans):
        wcb0 = colbase
        nb_list = []
        for b, calls in blocks:
            ncalls = []
            for cl in calls:
                n_idx = len(cl) * P
                slot0 = chunk_meta[cl[0]]["c0"]
                call_list.append((colbase, slot0, n_idx))
                ncalls.append((colbase, cl))
                colbase += n_idx // 16
            nb_list.append((b, ncalls))
        new_plans.append(nb_list)
        wave_colspan.append((wcb0, colbase))
    wave_plans = new_plans
    IDXCOLS = colbase
    idx_dev = np.zeros((NC, P, IDXCOLS), np.int16)
    for cb, slot0, n_idx in call_list:
        seg = idx_flat[:, slot0:slot0 + n_idx]
        wrap = seg.reshape(NC, n_idx // 16, 16).transpose(0, 2, 1)
        idx_dev[:, :, cb:cb + n_idx // 16] = np.tile(wrap, (1, 8, 1))

    # ---- dense inputs
    xg = np.zeros((NPAD, D), np.float16)
    xg[:N] = x.astype(np.float16)
    wt4 = np.empty((P, NREL * D), np.float16)
    for r in range(NREL):
        wt4[:, r * D:(r + 1) * D] = W_nei[r].T.astype(np.float16)
    wselft = W_self.sum(axis=0).T.astype(np.float16).copy()
    bsum = b_self.sum(axis=0).astype(np.float32)
    iotaf = np.tile(np.arange(OHW_MAX, dtype=np.float16), (P, 1))

    # xtp per core: [NC, 128, T_CORE*128] column (j*128+p) = x[node(j,p)]
    node_at = np.full((NC, T_CORE, P), -1, np.int64)
    node_at[core_of_tile[tile_of], local_of_tile[tile_of], slot_of] = np.arange(N)
    xtp = np.zeros((NC, D, T_CORE * P), np.float16)
    for c in range(NC):
        nn = node_at[c].reshape(-1)
        valid = nn >= 0
        xtp[c][:, valid] = x[nn[valid]].T.astype(np.float16)

    s = Sched()
    s.N, s.E, s.NPAD, s.NT, s.T_CORE, s.NW = N, E, NPAD, NT, T_CORE, NW
    s.CH, s.IDXCOLS = CH, IDXCOLS
    s.total_slots = total_slots
    s.waves = waves
    s.wave_plans = wave_plans
    s.wave_info = wave_info
    s.wave_colspan = wave_colspan
    s.chunk_meta = chunk_meta
    s.core_of_tile, s.local_of_tile = core_of_tile, local_of_tile
    s.tile_of, s.slot_of = tile_of, slot_of
    s.in_shared = dict(xg=xg, wt4=wt4, wselft=wselft, iotaf=iotaf)
    s.bsum = bsum
    s.in_percore = [
        dict(idx16=idx_dev[c], dkew=dkew[c], xtp=xtp[c]) for c in range(NC)
    ]
    return s


# ----------------------------------------------------------------- bass build
def build_bass(s, num_devices=NC):
    f16 = mybir.dt.float16
    f32 = mybir.dt.float32
    i16 = mybir.dt.int16

    nc = bacc.Bacc("TRN2", num_devices=num_devices)
    xg = nc.dram_tensor("xg", [s.NPAD, D], f16, kind="ExternalInput")
    wt4 = nc.dram_tensor("wt4", [P, NREL * D], f16, kind="ExternalInput")
    wselft = nc.dram_tensor("wselft", [P, D], f16, kind="ExternalInput")
    iotaf = nc.dram_tensor("iotaf", [P, OHW_MAX], f16, kind="ExternalInput")
    xtp = nc.dram_tensor("xtp", [P, s.T_CORE * P], f16, kind="ExternalInput")
    idx16 = nc.dram_tensor("idx16", [P, s.IDXCOLS], i16, kind="ExternalInput")
    dkew = nc.dram_tensor("dkew", [P, 2 * s.CH], f32, kind="ExternalInput")
    outT = nc.dram_tensor("outT", [P, s.T_CORE * P], f16, kind="ExternalOutput")

    nc.gpsimd.load_library(library_config.mlp)
    with tile.TileContext(nc) as tc:
        with (
            tc.tile_pool(name="const", bufs=1) as cpool,
            tc.tile_pool(name="meta", bufs=2) as mpool,
            tc.tile_pool(name="g", bufs=GBUFS) as gpool,
            tc.tile_pool(name="oh", bufs=OHBUFS) as ohpool,
            tc.tile_pool(name="agg", bufs=ABUFS) as apool,
            tc.tile_pool(name="st", bufs=SBUFS) as spool,
            tc.tile_pool(name="p2", bufs=1, space="PSUM") as p2pool,
        ):
            wt4_t = cpool.tile([P, NREL * D], f16)
            nc.sync.dma_start(out=wt4_t[:], in_=wt4[:, :])
            wself_t = cpool.tile([P, D], f16)
            nc.sync.dma_start(out=wself_t[:], in_=wselft[:, :])
            iota_t = cpool.tile([P, OHW_MAX], f16)
            nc.sync.dma_start(out=iota_t[:], in_=iotaf[:, :])

            banks = [
                p2pool.tile([P, NREL * P], f32, space="PSUM", tag=f"bank{k}",
                            name=f"bank{k}")
                for k in range(WAVE_T)
            ]

            for wi, blocks, (wcb0, wcb1) in zip(
                s.wave_info, s.wave_plans, s.wave_colspan
            ):
                w, jlo, wsz, ch0, nchw = (
                    wi["w"], wi["jlo"], wi["wsz"], wi["ch0"], wi["nch"]
                )
                idx_w = mpool.tile([P, max(wcb1 - wcb0, 1)], i16, tag="idxw")
                nc.sync.dma_start(out=idx_w[:], in_=idx16[:, wcb0:wcb1])
                dkew_w = mpool.tile([P, max(2 * nchw, 1)], f32, tag="dkew")
                nc.sync.dma_start(
                    out=dkew_w[:], in_=dkew[:, 2 * ch0:2 * (ch0 + nchw)]
                )
                xtp_w = mpool.tile([P, wsz * P], f16, tag="xtpw")
                nc.sync.dma_start(
                    out=xtp_w[:], in_=xtp[:, jlo * P:(jlo + wsz) * P]
                )

                for b, calls in blocks:
                    lo = b * BLKW
                    hi = lo + BLKW
                    for cb, cl in calls:
                        nch = len(cl)
                        g_t = gpool.tile([P, nch, D], f16, tag="g")
                        nc.gpsimd.dma_gather(
                            out_ap=g_t[:],
                            in_ap=xg[lo:hi, :],
                            idxs_ap=idx_w[:, cb - wcb0:cb - wcb0 + nch * 8],
                            num_idxs=nch * P,
                            num_idxs_reg=nch * P,
                            elem_size=D,
                            single_packet=False,
                        )
                        for pos, ci in enumerate(cl):
                            cm = s.chunk_meta[ci]
                            wq = cm["wq"]
                            oh = ohpool.tile([P, OHW_MAX], f16, tag="oh")
                            nc.vector.tensor_scalar(
                                out=oh[:, :wq * P],
                                in0=iota_t[:, :wq * P],
                                scalar1=dkew_w[:, 2 * (ci - ch0):2 * (ci - ch0) + 1],
                                scalar2=dkew_w[:, 2 * (ci - ch0) + 1:2 * (ci - ch0) + 2],
                                op0=mybir.AluOpType.is_equal,
                                op1=mybir.AluOpType.mult,
                            )
                            for (j, r, ohq, st, sp) in cm["mms"]:
                                bank = banks[(j - jlo) % WAVE_T]
                                nc.tensor.matmul(
                                    out=bank[:, r * P:(r + 1) * P],
                                    lhsT=g_t[:, pos, :],
                                    rhs=oh[:, ohq * P:(ohq + 1) * P],
                                    start=bool(st),
                                    stop=bool(sp),
                                    skip_group_check=True,
                                )

                # per-tile finalize: copy AGGT, reuse bank quarter 0 as out
                stage = spool.tile([P, wsz * P], f16, tag="stage")
                for j in range(jlo, jlo + wsz):
                    bank = banks[(j - jlo) % WAVE_T]
                    aggsb = apool.tile([P, NREL * P], f16, tag="agg")
                    nc.scalar.copy(out=aggsb[:], in_=bank[:])
                    outq = bank[:, 0:P]
                    nc.tensor.matmul(
                        out=outq,
                        lhsT=xtp_w[:, (j - jlo) * P:(j - jlo + 1) * P],
                        rhs=wself_t[:],
                        start=True,
                        stop=False,
                        skip_group_check=True,
                    )
                    for r in range(NREL):
                        nc.tensor.matmul(
                            out=outq,
                            lhsT=aggsb[:, r * P:(r + 1) * P],
                            rhs=wt4_t[:, r * P:(r + 1) * P],
                            start=False,
                            stop=(r == NREL - 1),
                            skip_group_check=True,
                        )
                    nc.scalar.copy(
                        out=stage[:, (j - jlo) * P:(j - jlo + 1) * P], in_=outq
                    )
                nc.sync.dma_start(
                    out=outT[:, jlo * P:(jlo + wsz) * P], in_=stage[:]
                )
    nc.compile()
    return nc


def kernel(**inputs):
    s = build_schedule(inputs)
    nc = build_bass(s)
    in_maps = []
    for c in range(NC):
        m = dict(s.in_shared)
        m.update(s.in_percore[c])
        in_maps.append(m)
    res = bass_utils.run_bass_kernel_spmd(nc, in_maps, core_ids=list(range(NC)))
    outT = np.stack([res.results[c]["outT"] for c in range(NC)])  # [NC,128,T*128]
    return assemble(s, outT)


def assemble(s, outT):
    # outT[c][p, j*128 + d] = out row of node at (core c, tile j, slot p)
    o4 = np.asarray(outT, np.float32).reshape(NC, P, s.T_CORE, D)
    nodes = np.arange(s.N)
    c = s.core_of_tile[s.tile_of[nodes]]
    t = s.local_of_tile[s.tile_of[nodes]]
    p = s.slot_of[nodes]
    return (o4[c, p, t, :] + s.bsum[None, :]).astype(np.float32)


# revision 9
# speedup vs baseline: 2.0265x; 1.0319x over previous
"""Trainium2 Bass kernel for nn_DictlessHeteroLayer (hetero GNN message passing).

  out = sum_r [ x @ W_self[r].T + b_self[r]
                + scatter_add_dst( ew * (x @ W_nei[r].T)[src] ) ]

Strategy (8 NeuronCores, SPMD, no collectives) — aggregate-first:
  By linearity, scatter_add_dst(ew * (x W_r^T)[src]) = (scatter_add_dst(ew *
  x[src])) W_r^T, so we aggregate RAW x rows per (dst tile, relation) and
  apply W_nei once per (tile, rel) at the end.  This removes the baseline's
  phase-1 H = x@W^T materialization (~128 MB of HBM traffic per core).

  * Host assigns dst nodes to 128-slot tiles (degree-balanced FFD), deals
    tiles to cores (edge balanced).  Each core fully owns its tiles' output
    rows -> no cross-core reduction; host re-assembles.
  * Edges sharded by dst-tile owner.  Per wave of 8 tiles (= 8 PSUM banks,
    bank b_t holds tile t's AGGT: [k=128, 4 rel * 128 dst slots] f32):
    for each of 4 equal 25088-row gather windows, a static (tile, rel)
    sub-cell layout (sizes = max over cores) is packed into 128-edge chunks.
    Per chunk: gpsimd dma_gather of 128 f16 x rows (256 B/row descriptors);
    DVE builds ONE shared one-hot OH[e, (q - qbase)*128 + slot] * ew (f16,
    4x DVE perf mode) covering all quarters the chunk spans; TensorE does one
    quarter-pure matmul per touched (tile, rel):
        AGGT[k, r*128 + p] += sum_e Xg[e, k] * OH[e, col]
    with exact first/last-touch start/stop per quarter.
  * Per tile: Act copies the bank to SBUF f16; the bank's first quarter is
    then reused as out accumulator: out[p, d] = xtp_tile @ WselfSum^T
    + sum_r AGGT_r^T ... via 5 matmuls (lhsT = xtp slice / aggsb quarter,
    rhs = WselfT / W_nei[r]^T, all f16); drained f16 to DRAM (bias added on
    host, output cast to f32 on host).
  * Duplicate (rel, src, dst) edges merged on host (weights summed, exact).
"""
import numpy as np

import concourse.bacc as bacc
import concourse.bass as bass
import concourse.mybir as mybir
import concourse.tile as tile
from concourse import bass_utils, library_config

P = 128
D = 128
NREL = 4
NC = 8
import os as _os
BLKW = 25088              # gather window rows = NPAD / 4
NBLK = 4
WAVE_T = int(_os.environ.get("KWAVET", "8"))    # tiles per wave = psum banks
MAX_CALL = int(_os.environ.get("KMAXCALL", "32"))  # chunks per dma_gather
GBUFS = int(_os.environ.get("KGBUFS", "6"))
OHBUFS = int(_os.environ.get("KOHBUFS", "12"))
ABUFS = int(_os.environ.get("KABUFS", "4"))
SBUFS = int(_os.environ.get("KSBUFS", "2"))
OHW_MAX = 1024            # max one-hot width (8 quarters)


class Sched:
    pass


def build_schedule(inputs):
    import ml_dtypes
    x = np.asarray(inputs["x"], np.float32)
    ei = np.asarray(inputs["edge_index"])
    ew = np.asarray(inputs["edge_weight"], np.float32)
    rel_ptr = np.asarray(inputs["rel_ptr"]).astype(np.int64)
    W_self = np.asarray(inputs["W_self"], np.float32)
    b_self = np.asarray(inputs["b_self"], np.float32)
    W_nei = np.asarray(inputs["W_nei"], np.float32)

    N = x.shape[0]
    E = ei.shape[1]
    NT0 = -(-N // P)
    T_CORE = -(-NT0 // NC)
    NT = T_CORE * NC
    NPAD = NT * P
    assert NPAD == NBLK * BLKW, (NPAD, NBLK * BLKW)

    src = ei[0].astype(np.int64)
    dst = ei[1].astype(np.int64)
    rel = (np.searchsorted(rel_ptr, np.arange(E), side="right") - 1).astype(np.int64)

    # merge duplicate (rel, src, dst) edges (sum their weights) — exact
    ukey = (rel * N + src) * N + dst
    uorder = np.argsort(ukey, kind="stable")
    uk = ukey[uorder]
    first = np.ones(E, bool)
    first[1:] = uk[1:] != uk[:-1]
    gids = np.cumsum(first) - 1
    ew_sum = np.zeros(int(gids[-1]) + 1, np.float64)
    np.add.at(ew_sum, gids, ew[uorder].astype(np.float64))
    keep = uorder[first]
    src, dst, rel = src[keep], dst[keep], rel[keep]
    ew = ew_sum.astype(np.float32)
    E = len(src)

    deg = np.bincount(dst, minlength=N)

    # ---- node -> (tile, slot): first-fit decreasing over NT tiles
    import heapq
    order = np.argsort(-deg, kind="stable")
    tile_of = np.empty(N, np.int64)
    slot_of = np.empty(N, np.int64)
    heap = [(0, t, 0) for t in range(NT)]
    heapq.heapify(heap)
    for n in order:
        load, t, used = heapq.heappop(heap)
        tile_of[n] = t
        slot_of[n] = used
        used += 1
        if used < P:
            heapq.heappush(heap, (load + int(deg[n]), t, used))

    tile_load = np.bincount(tile_of[dst], minlength=NT)

    # ---- tiles -> (core, local slot j).  The static sub-cell sizes are
    # max-over-cores of the 8 tiles sharing a j-slot, so group tiles with
    # SIMILAR (rel, window) count profiles into each j-slot (greedy set
    # cover minimizing sum-of-elementwise-max growth), then deal each
    # group's tiles to cores balancing total core load.
    Vt = np.zeros((NT, NREL * NBLK), np.int64)
    np.add.at(Vt, (tile_of[dst], rel * NBLK + (src // BLKW)), 1)
    t_order = np.argsort(-tile_load, kind="stable")
    unassigned = np.ones(NT, bool)
    groups = []
    for g in range(T_CORE):
        seed = next(t for t in t_order if unassigned[t])
        unassigned[seed] = False
        m = Vt[seed].astype(np.float64).copy()
        members = [seed]
        for _ in range(NC - 1):
            cand = np.nonzero(unassigned)[0]
            growth = np.maximum(Vt[cand] - m, 0).sum(axis=1)
            pick = cand[np.argmin(growth * 1e6 - tile_load[cand])]
            unassigned[pick] = False
            m = np.maximum(m, Vt[pick])
            members.append(int(pick))
        groups.append(members)
    core_of_tile = np.empty(NT, np.int64)
    local_of_tile = np.empty(NT, np.int64)
    core_load = np.zeros(NC, np.int64)
    for g, members in enumerate(groups):
        mem = sorted(members, key=lambda t: -tile_load[t])
        corder = np.argsort(core_load, kind="stable")
        for t, c in zip(mem, corder):
            core_of_tile[t] = c
            local_of_tile[t] = g
            core_load[c] += tile_load[t]

    # ---- per-edge attributes
    e_tile = tile_of[dst]
    e_core = core_of_tile[e_tile]
    e_j = local_of_tile[e_tile]              # local tile 0..T_CORE-1
    e_b = src // BLKW                        # gather window
    e_slot = slot_of[dst]

    # ---- static sub-cell sizes: s[j, r, b] = max over cores
    cnt = np.zeros((NC, T_CORE, NREL, NBLK), np.int64)
    np.add.at(cnt, (e_core, e_j, rel, e_b), 1)
    scell = cnt.max(axis=0)                  # [T_CORE, NREL, NBLK]
    assert scell.reshape(T_CORE, -1).sum(axis=1).min() > 0

    # ---- waves of WAVE_T tiles; per (wave, b) segment: pack sub-cells
    # (j-major, then rel), pad segment to x128.  Chunk = 128 slots.
    waves = []
    j0 = 0
    while j0 < T_CORE:
        waves.append((j0, min(WAVE_T, T_CORE - j0)))
        j0 += WAVE_T
    NW = len(waves)

    cell_off = np.zeros((T_CORE, NREL, NBLK), np.int64)
    off = 0
    # per chunk: (b, qbase, width_q, mms); mms = [(j, r, ohq, start, stop)]
    chunk_meta = []
    wave_plans = []      # per wave: dict(b -> list of calls; call = [chunk ids])
    wave_info = []
    first_touch = {}
    last_touch = {}
    for w, (jlo, wsz) in enumerate(waves):
        wave_ch0 = len(chunk_meta)
        woff0 = off
        blocks = []
        for b in range(NBLK):
            seg0 = off
            ranges = []                      # (q_local, j, r, lo, hi)
            for j in range(jlo, jlo + wsz):
                for r in range(NREL):
                    sz = int(scell[j, r, b])
                    if sz == 0:
                        continue
                    cell_off[j, r, b] = off
                    q = (j - jlo) * NREL + r
                    ranges.append((q, j, r, off, off + sz))
                    off += sz
            seg_edges = off - seg0
            nch = -(-seg_edges // P) if seg_edges else 0
            off = seg0 + nch * P
            # chunks of this segment
            ch_ids = []
            for k in range(nch):
                c0, c1 = seg0 + k * P, seg0 + (k + 1) * P
                touch = [rg for rg in ranges if rg[4] > c0 and rg[3] < c1]
                qbase = touch[0][0] if touch else 0
                qmax = touch[-1][0] if touch else 0
                width_q = qmax - qbase + 1
                assert width_q * P <= OHW_MAX, width_q
                mms = []
                for (q, j, r, lo, hi) in touch:
                    # PSUM semantics: start=True clears has_written for the
                    # WHOLE bank; later matmuls (start=False) init-or-accum
                    # per element.  So start only on the first matmul into
                    # tile j's bank this wave, stop on the last.
                    st = j not in first_touch
                    first_touch.setdefault(j, len(chunk_meta))
                    last_touch[j] = (len(chunk_meta), len(mms))
                    mms.append([j, r, q - qbase, st, False])
                ch_ids.append(len(chunk_meta))
                chunk_meta.append(dict(b=b, qbase=qbase, wq=width_q, mms=mms,
                                       w=w, c0=c0))
            calls = []
            pos = 0
            while pos < len(ch_ids):
                n = min(MAX_CALL, len(ch_ids) - pos)
                calls.append(ch_ids[pos:pos + n])
                pos += n
            blocks.append((b, calls))
        wave_plans.append(blocks)
        wave_info.append(dict(w=w, jlo=jlo, wsz=wsz, ch0=wave_ch0,
                              nch=len(chunk_meta) - wave_ch0, off0=woff0,
                              off1=off))
    # stop flags
    for j, (ci, mi) in last_touch.items():
        chunk_meta[ci]["mms"][mi][4] = True
    # every tile must be touched (else final matmuls read garbage)
    for j in range(T_CORE):
        assert j in first_touch, j
    # every (j, r) quarter must be written at least once (has_written init);
    # quarters with no edges at all would leave stale psum.
    qtouch = set()
    for cm in chunk_meta:
        for (j, r, _, _, _) in cm["mms"]:
            qtouch.add((j, r))
    for j in range(T_CORE):
        for r in range(NREL):
            assert (j, r) in qtouch, (j, r)

    CH = len(chunk_meta)
    total_slots = off
    assert total_slots == CH * P

    # ---- per-core flat edge arrays in schedule order
    key = ((e_core * T_CORE + e_j) * NREL + rel) * NBLK + e_b
    sort_idx = np.lexsort((src, key))
    skey = key[sort_idx]
    newg = np.ones(E, bool)
    newg[1:] = skey[1:] != skey[:-1]
    group_first = np.nonzero(newg)[0]
    group_id = np.cumsum(newg) - 1
    rank = np.arange(E) - group_first[group_id]

    se = sort_idx
    pos_in_core = cell_off[e_j[se], rel[se], e_b[se]] + rank
    core_se = e_core[se]

    idx_flat = np.zeros((NC, total_slots), np.int16)
    dk_flat = np.zeros((NC, total_slots), np.float32)
    ew_flat = np.zeros((NC, total_slots), np.float32)
    idx_flat[core_se, pos_in_core] = (src[se] - e_b[se] * BLKW).astype(np.int16)
    # dstkey = (q_local - qbase_of_chunk)*128 + slot
    q_of_edge = ((e_j[se] - (e_j[se] // WAVE_T) * WAVE_T) * NREL + rel[se])
    ch_of_pos = pos_in_core // P
    qbase_arr = np.asarray([cm["qbase"] for cm in chunk_meta], np.int64)
    dk_flat[core_se, pos_in_core] = (
        (q_of_edge - qbase_arr[ch_of_pos]) * P + e_slot[se]
    ).astype(np.float32)
    ew_flat[core_se, pos_in_core] = ew[se]

    # ---- device metadata
    # dkew: [NC, 128, 2*CH]  (col 2c = dstkey, col 2c+1 = ew)
    dkew = np.zeros((NC, P, 2 * CH), np.float32)
    dk3 = dk_flat.reshape(NC, CH, P).transpose(0, 2, 1)
    ew3 = ew_flat.reshape(NC, CH, P).transpose(0, 2, 1)
    dkew[:, :, 0::2] = dk3
    dkew[:, :, 1::2] = ew3

    # idx16 wrapped per call: [NC, 128, IDXCOLS]
    call_list = []           # (colbase, slot0, n_idx) per call
    colbase = 0
    new_plans = []
    wave_colspan = []
    for w, blocks in enumerate(wave_plans):
        wcb0 = colbase
        nb_list = []
        for b, calls in blocks:
            ncalls = []
            for cl in calls:
                n_idx = len(cl) * P
                slot0 = chunk_meta[cl[0]]["c0"]
                call_list.append((colbase, slot0, n_idx))
                ncalls.append((colbase, cl))
                colbase += n_idx // 16
            nb_list.append((b, ncalls))
        new_plans.append(nb_list)
        wave_colspan.append((wcb0, colbase))
    wave_plans = new_plans
    IDXCOLS = colbase
    idx_dev = np.zeros((NC, P, IDXCOLS), np.int16)
    for cb, slot0, n_idx in call_list:
        seg = idx_flat[:, slot0:slot0 + n_idx]
        wrap = seg.reshape(NC, n_idx // 16, 16).transpose(0, 2, 1)
        idx_dev[:, :, cb:cb + n_idx // 16] = np.tile(wrap, (1, 8, 1))

    # ---- dense inputs
    xg = np.zeros((NPAD, D), np.float16)
    xg[:N] = x.astype(np.float16)
    wt4 = np.empty((P, NREL * D), np.float16)
    for r in range(NREL):
        wt4[:, r * D:(r + 1) * D] = W_nei[r].T.astype(np.float16)
    wselft = W_self.sum(axis=0).T.astype(np.float16).copy()
    bsum = b_self.sum(axis=0).astype(np.float32)
    iotaf = np.tile(np.arange(OHW_MAX, dtype=np.float16), (P, 1))

    # xtp per core: [NC, 128, T_CORE*128] column (j*128+p) = x[node(j,p)]
    node_at = np.full((NC, T_CORE, P), -1, np.int64)
    node_at[core_of_tile[tile_of], local_of_tile[tile_of], slot_of] = np.arange(N)
    xtp = np.zeros((NC, D, T_CORE * P), np.float16)
    for c in range(NC):
        nn = node_at[c].reshape(-1)
        valid = nn >= 0
        xtp[c][:, valid] = x[nn[valid]].T.astype(np.float16)

    s = Sched()
    s.N, s.E, s.NPAD, s.NT, s.T_CORE, s.NW = N, E, NPAD, NT, T_CORE, NW
    s.CH, s.IDXCOLS = CH, IDXCOLS
    s.total_slots = total_slots
    s.waves = waves
    s.wave_plans = wave_plans
    s.wave_info = wave_info
    s.wave_colspan = wave_colspan
    s.chunk_meta = chunk_meta
    s.core_of_tile, s.local_of_tile = core_of_tile, local_of_tile
    s.tile_of, s.slot_of = tile_of, slot_of
    s.in_shared = dict(xg=xg, wt4=wt4, wselft=wselft, iotaf=iotaf)
    s.bsum = bsum
    s.in_percore = [
        dict(idx16=idx_dev[c], dkew=dkew[c], xtp=xtp[c]) for c in range(NC)
    ]
    return s


# ----------------------------------------------------------------- bass build
def build_bass(s, num_devices=NC):
    f16 = mybir.dt.float16
    f32 = mybir.dt.float32
    i16 = mybir.dt.int16

    nc = bacc.Bacc("TRN2", num_devices=num_devices)
    xg = nc.dram_tensor("xg", [s.NPAD, D], f16, kind="ExternalInput")
    wt4 = nc.dram_tensor("wt4", [P, NREL * D], f16, kind="ExternalInput")
    wselft = nc.dram_tensor("wselft", [P, D], f16, kind="ExternalInput")
    iotaf = nc.dram_tensor("iotaf", [P, OHW_MAX], f16, kind="ExternalInput")
    xtp = nc.dram_tensor("xtp", [P, s.T_CORE * P], f16, kind="ExternalInput")
    idx16 = nc.dram_tensor("idx16", [P, s.IDXCOLS], i16, kind="ExternalInput")
    dkew = nc.dram_tensor("dkew", [P, 2 * s.CH], f32, kind="ExternalInput")
    outT = nc.dram_tensor("outT", [P, s.T_CORE * P], f16, kind="ExternalOutput")

    nc.gpsimd.load_library(library_config.mlp)
    with tile.TileContext(nc) as tc:
        with (
            tc.tile_pool(name="const", bufs=1) as cpool,
            tc.tile_pool(name="meta", bufs=2) as mpool,
            tc.tile_pool(name="g", bufs=GBUFS) as gpool,
            tc.tile_pool(name="oh", bufs=OHBUFS) as ohpool,
            tc.tile_pool(name="agg", bufs=ABUFS) as apool,
            tc.tile_pool(name="st", bufs=SBUFS) as spool,
            tc.tile_pool(name="p2", bufs=1, space="PSUM") as p2pool,
        ):
            wt4_t = cpool.tile([P, NREL * D], f16)
            nc.sync.dma_start(out=wt4_t[:], in_=wt4[:, :])
            wself_t = cpool.tile([P, D], f16)
            nc.sync.dma_start(out=wself_t[:], in_=wselft[:, :])
            iota_t = cpool.tile([P, OHW_MAX], f16)
            nc.sync.dma_start(out=iota_t[:], in_=iotaf[:, :])

            banks = [
                p2pool.tile([P, NREL * P], f32, space="PSUM", tag=f"bank{k}",
                            name=f"bank{k}")
                for k in range(WAVE_T)
            ]

            for wi, blocks, (wcb0, wcb1) in zip(
                s.wave_info, s.wave_plans, s.wave_colspan
            ):
                w, jlo, wsz, ch0, nchw = (
                    wi["w"], wi["jlo"], wi["wsz"], wi["ch0"], wi["nch"]
                )
                idx_w = mpool.tile([P, max(wcb1 - wcb0, 1)], i16, tag="idxw")
                nc.sync.dma_start(out=idx_w[:], in_=idx16[:, wcb0:wcb1])
                dkew_w = mpool.tile([P, max(2 * nchw, 1)], f32, tag="dkew")
                nc.sync.dma_start(
                    out=dkew_w[:], in_=dkew[:, 2 * ch0:2 * (ch0 + nchw)]
                )
                xtp_w = mpool.tile([P, wsz * P], f16, tag="xtpw")
                nc.sync.dma_start(
                    out=xtp_w[:], in_=xtp[:, jlo * P:(jlo + wsz) * P]
                )

                for b, calls in blocks:
                    lo = b * BLKW
                    hi = lo + BLKW
                    for cb, cl in calls:
                        nch = len(cl)
                        g_t = gpool.tile([P, nch, D], f16, tag="g")
                        nc.gpsimd.dma_gather(
                            out_ap=g_t[:],
                            in_ap=xg[lo:hi, :],
                            idxs_ap=idx_w[:, cb - wcb0:cb - wcb0 + nch * 8],
                            num_idxs=nch * P,
                            num_idxs_reg=nch * P,
                            elem_size=D,
                            single_packet=False,
                        )
                        for pos, ci in enumerate(cl):
                            cm = s.chunk_meta[ci]
                            wq = cm["wq"]
                            oh = ohpool.tile([P, OHW_MAX], f16, tag="oh")
                            nc.vector.tensor_scalar(
                                out=oh[:, :wq * P],
                                in0=iota_t[:, :wq * P],
                                scalar1=dkew_w[:, 2 * (ci - ch0):2 * (ci - ch0) + 1],
                                scalar2=dkew_w[:, 2 * (ci - ch0) + 1:2 * (ci - ch0) + 2],
                                op0=mybir.AluOpType.is_equal,
                                op1=mybir.AluOpType.mult,
                            )
                            for (j, r, ohq, st, sp) in cm["mms"]:
                                bank = banks[(j - jlo) % WAVE_T]
                                nc.tensor.matmul(
                                    out=bank[:, r * P:(r + 1) * P],
                                    lhsT=g_t[:, pos, :],
                                    rhs=oh[:, ohq * P:(ohq + 1) * P],
                                    start=bool(st),
                                    stop=bool(sp),
                                    skip_group_check=True,
                                )

                # per-tile finalize: copy AGGT, reuse bank quarter 0 as out
                stage = spool.tile([P, wsz * P], f16, tag="stage")
                for j in range(jlo, jlo + wsz):
                    bank = banks[(j - jlo) % WAVE_T]
                    aggsb = apool.tile([P, NREL * P], f16, tag="agg")
                    nc.scalar.copy(out=aggsb[:], in_=bank[:])
                    outq = bank[:, 0:P]
                    nc.tensor.matmul(
                        out=outq,
                        lhsT=xtp_w[:, (j - jlo) * P:(j - jlo + 1) * P],
                        rhs=wself_t[:],
                        start=True,
                        stop=False,
                        skip_group_check=True,
                    )
                    for r in range(NREL):
                        nc.tensor.matmul(
                            out=outq,
                            lhsT=aggsb[:, r * P:(r + 1) * P],
                            rhs=wt4_t[:, r * P:(r + 1) * P],
                            start=False,
                            stop=(r == NREL - 1),
                            skip_group_check=True,
                        )
                    nc.scalar.copy(
                        out=stage[:, (j - jlo) * P:(j - jlo + 1) * P], in_=outq
                    )
                nc.sync.dma_start(
                    out=outT[:, jlo * P:(jlo + wsz) * P], in_=stage[:]
                )
    nc.compile()
    return nc


def kernel(**inputs):
    s = build_schedule(inputs)
    nc = build_bass(s)
    in_maps = []
    for c in range(NC):
        m = dict(s.in_shared)
        m.update(s.in_percore[c])
        in_maps.append(m)
    res = bass_utils.run_bass_kernel_spmd(nc, in_maps, core_ids=list(range(NC)))
    outT = np.stack([res.results[c]["outT"] for c in range(NC)])  # [NC,128,T*128]
    return assemble(s, outT)


def assemble(s, outT):
    # outT[c][p, j*128 + d] = out row of node at (core c, tile j, slot p)
    o4 = np.asarray(outT, np.float32).reshape(NC, P, s.T_CORE, D)
    nodes = np.arange(s.N)
    c = s.core_of_tile[s.tile_of[nodes]]
    t = s.local_of_tile[s.tile_of[nodes]]
    p = s.slot_of[nodes]
    return (o4[c, p, t, :] + s.bsum[None, :]).astype(np.float32)
